# revision 5
# baseline (speedup 1.0000x reference)
"""PowerWhitening Trainium2 kernel.

Computes, for x [524288, 128] and R [128, 128] (both fp32):
  xc = x - mean(x, axis=0)
  C  = xc^T xc / N
  128 sequential power-iteration/deflation steps (100 iters each, restructured
  as C^100 = C32·C32·C32·C4 via repeated squaring; per-step normalization is
  algebraically redundant and folded into the final scalars)
  out = xc @ W^T

Distribution: data-parallel over rows across 8 NeuronCores; the [128,129]
(x^T x | colsum) statistics are AllReduced; the deflation loop is replicated
on every core; each core applies its own row shard.
"""

import sys
import os

sys.path.insert(0, "/opt/trn_rl_repo")

import numpy as np
import concourse.bass as bass
import concourse.bacc as bacc
import concourse.tile as tile
import concourse.mybir as mybir
from concourse import bass_utils
from concourse.masks import make_identity

P = 128          # partitions == feature dim D
D = 128
N_TOTAL = 524288
NCORES = 8
SHARD = N_TOTAL // NCORES          # 65536 rows per core
SUPER = 1024                       # rows per DMA super-tile
NSUP = SHARD // SUPER              # 64 super-tiles
BLKS = SUPER // P                  # 8 row-blocks per super-tile
NSTEPS = 128                       # deflation steps

F32 = mybir.dt.float32
ALU = mybir.AluOpType
AX = mybir.AxisListType


def _split_copy(nc, dst, src):
    """PSUM->SBUF copy of a [128, 128] tile split across DVE and ACT."""
    nc.vector.tensor_copy(dst[:, 0:64], src[:, 0:64])
    nc.scalar.copy(dst[:, 64:128], src[:, 64:128])


def build_nc():
    nc = bacc.Bacc("TRN2", target_bir_lowering=False, debug=False,
                   enable_asserts=True, num_devices=NCORES)
    xs_in = nc.dram_tensor("xs", [SHARD, D], F32, kind="ExternalInput").ap()
    r_in = nc.dram_tensor("R", [D, D], F32, kind="ExternalInput").ap()
    out_d = nc.dram_tensor("out", [SHARD, D], F32, kind="ExternalOutput").ap()

    with tile.TileContext(nc) as tc:
        with tc.tile_pool(name="consts", bufs=1) as consts, \
             tc.tile_pool(name="dram", bufs=1, space="DRAM") as dram:
            ident = consts.tile([P, P], F32)
            make_identity(nc, ident)
            ones = consts.tile([P, P], F32)
            nc.vector.memset(ones[:], 1.0)
            rcols = consts.tile([D, D], F32)
            nc.sync.dma_start(rcols[:], r_in[:])
            mu_col = consts.tile([P, 1], F32)
            wacc = consts.tile([P, P], F32)
            nc.vector.memset(wacc[:], 0.0)
            wpad = consts.tile([P, P], F32)
            nc.vector.memset(wpad[:], 0.0)
            stats_sb = consts.tile([P, D + 1], F32)

            # ---------------- Phase A: stats pass ----------------
            with tc.tile_pool(name="xa", bufs=3) as xa_pool, \
                 tc.tile_pool(name="psA", bufs=1, space="PSUM") as psA:
                stats_ps = psA.tile([P, D + 1], F32, tag="stats")
                for i in range(NSUP):
                    xa = xa_pool.tile([P, BLKS, D + 1], F32, tag="xa")
                    nc.sync.dma_start(
                        xa[:, :, 0:D],
                        xs_in[i * SUPER:(i + 1) * SUPER, :].rearrange(
                            "(n p) d -> p n d", p=P),
                    )
                    nc.gpsimd.memset(xa[:, :, D:D + 1], 1.0)
                    for n in range(BLKS):
                        nc.tensor.matmul(
                            stats_ps[:],
                            lhsT=xa[:, n, 0:D],
                            rhs=xa[:, n, 0:D + 1],
                            start=(i == 0 and n == 0),
                            stop=(i == NSUP - 1 and n == BLKS - 1),
                        )
                _split_copy(nc, stats_sb[:, 0:D], stats_ps[:, 0:D])
                nc.vector.tensor_copy(stats_sb[:, D:D + 1], stats_ps[:, D:D + 1])

            # AllReduce the [128, 129] stats across all 8 cores.
            cc_in = dram.tile([P, D + 1], F32)
            cc_out = dram.tile([P, D + 1], F32)
            nc.sync.dma_start(cc_in[:], stats_sb[:])
            nc.gpsimd.collective_compute(
                "AllReduce", ALU.add,
                replica_groups=[list(range(NCORES))],
                ins=[cc_in.opt()], outs=[cc_out.opt()],
            )
            stats_g = consts.tile([P, D + 1], F32)
            nc.sync.dma_start(stats_g[:], cc_out[:])

            # ---------------- Phase B: covariance + eigensolve ----------------
            with tc.tile_pool(name="cpool", bufs=2) as cpool, \
                 tc.tile_pool(name="sq", bufs=2) as sq, \
                 tc.tile_pool(name="vec", bufs=2) as vec, \
                 tc.tile_pool(name="psB", bufs=2, space="PSUM") as psB, \
                 tc.tile_pool(name="psV", bufs=2, space="PSUM") as psV, \
                 tc.tile_pool(name="psR", bufs=1, space="PSUM") as psR, \
                 tc.tile_pool(name="psN", bufs=1, space="PSUM") as psN, \
                 tc.tile_pool(name="psW", bufs=2, space="PSUM") as psW:

                # mu and C0 = stats/N - mu mu^T.  All outer products go
                # through the ones-matmul broadcast trick: fp32 matmuls with
                # K<128 or M=1 weights return zeros on TRN2 HW.
                nc.vector.tensor_scalar_mul(mu_col[:], stats_g[:, D:D + 1],
                                            1.0 / N_TOTAL)
                pmr = psR.tile([1, P], F32, tag="prow")
                nc.tensor.transpose(pmr[:], mu_col[:], ident[:])
                nc.vector.tensor_copy(wpad[0:1, :], pmr[:])
                pmrep = psW.tile([P, P], F32, tag="pwrep")
                nc.tensor.matmul(pmrep[:], lhsT=ones[:], rhs=wpad[:],
                                 start=True, stop=True)
                neg_mu = vec.tile([P, 1], F32, tag="negmu")
                nc.vector.tensor_scalar_mul(neg_mu[:], mu_col[:], -1.0)
                sc_stats = vec.tile([P, P], F32, tag="scstats")
                nc.vector.tensor_scalar_mul(sc_stats[:], stats_g[:, 0:D],
                                            1.0 / N_TOTAL)
                ccur = cpool.tile([P, P], F32, tag="C")
                nc.vector.scalar_tensor_tensor(
                    out=ccur[:], in0=pmrep[:], scalar=neg_mu[:],
                    in1=sc_stats[:], op0=ALU.mult, op1=ALU.add)

                for k in range(NSTEPS):
                    # --- squarings C2..C32 (+t1 = C4 v0 in PE slack) ---
                    mats = []
                    src = ccur
                    for si in range(5):
                        pm = psB.tile([P, P], F32, tag="pbig")
                        nc.tensor.matmul(pm[:], lhsT=src[:], rhs=src[:],
                                         start=True, stop=True)
                        ms = sq.tile([P, P], F32, tag=f"sq{si}")
                        _split_copy(nc, ms[:], pm[:])
                        mats.append(ms)
                        src = ms
                        if si == 1:
                            pt1 = psV.tile([P, 1], F32, tag="pvec")
                            nc.tensor.matmul(pt1[:], lhsT=mats[1][:],
                                             rhs=rcols[:, k:k + 1],
                                             start=True, stop=True)
                            t1 = vec.tile([P, 1], F32, tag="t1")
                            nc.vector.tensor_copy(t1[:], pt1[:])
                    c32 = mats[4]

                    # t2 = C32 t1 ; C33 = C * C32 in PE slack
                    pt2 = psV.tile([P, 1], F32, tag="pvec")
                    nc.tensor.matmul(pt2[:], lhsT=c32[:], rhs=t1[:],
                                     start=True, stop=True)
                    pc33 = psB.tile([P, P], F32, tag="pbig")
                    nc.tensor.matmul(pc33[:], lhsT=ccur[:], rhs=c32[:],
                                     start=True, stop=True)
                    t2 = vec.tile([P, 1], F32, tag="t2")
                    nc.vector.tensor_copy(t2[:], pt2[:])
                    c33 = sq.tile([P, P], F32, tag="sqc33")
                    _split_copy(nc, c33[:], pc33[:])

                    # t3 = C32 t2
                    pt3 = psV.tile([P, 1], F32, tag="pvec")
                    nc.tensor.matmul(pt3[:], lhsT=c32[:], rhs=t2[:],
                                     start=True, stop=True)
                    t3 = vec.tile([P, 1], F32, tag="t3")
                    nc.vector.tensor_copy(t3[:], pt3[:])

                    # w = C32 t3 = C^100 v0 ; u = C33 t3 = C w  (columns)
                    pwv = psV.tile([P, 1], F32, tag="pvec")
                    nc.tensor.matmul(pwv[:], lhsT=c32[:], rhs=t3[:],
                                     start=True, stop=True)
                    puv = psV.tile([P, 1], F32, tag="pvec")
                    nc.tensor.matmul(puv[:], lhsT=c33[:], rhs=t3[:],
                                     start=True, stop=True)
                    wu = vec.tile([P, 2], F32, tag="wu")
                    nc.vector.tensor_copy(wu[:, 0:1], pwv[:])
                    nc.scalar.copy(wu[:, 1:2], puv[:])

                    # replicated norms: n2[:,0]=||w||^2, n2[:,1]=||u||^2
                    wu2 = vec.tile([P, 2], F32, tag="wu2")
                    nc.vector.tensor_mul(wu2[:], wu[:], wu[:])
                    pn2 = psN.tile([P, 2], F32, tag="pn2")
                    nc.tensor.matmul(pn2[:], lhsT=ones[:], rhs=wu2[:],
                                     start=True, stop=True)
                    n2 = vec.tile([P, 2], F32, tag="n2")
                    nc.vector.tensor_copy(n2[:], pn2[:])

                    # w_row -> wpad row 0 -> W_rep (all partitions = w^T)
                    pwr = psR.tile([1, P], F32, tag="prow")
                    nc.tensor.transpose(pwr[:], wu[:, 0:1], ident[:])
                    nc.scalar.copy(wpad[0:1, :], pwr[:])
                    pwrep = psW.tile([P, P], F32, tag="pwrep")
                    nc.tensor.matmul(pwrep[:], lhsT=ones[:], rhs=wpad[:],
                                     start=True, stop=True)

                    # replicated scalars: inv=1/n2w, e=sqrt(n2u*inv),
                    # na=-e*inv, naw=-alpha*w
                    inv = vec.tile([P, 1], F32, tag="inv")
                    nc.vector.reciprocal(inv[:], n2[:, 0:1])
                    e_t = vec.tile([P, 1], F32, tag="e")
                    nc.scalar.activation(e_t[:], n2[:, 1:2],
                                         mybir.ActivationFunctionType.Sqrt,
                                         scale=inv[:])
                    na = vec.tile([P, 1], F32, tag="na")
                    nc.vector.tensor_scalar(out=na[:], in0=e_t[:],
                                            scalar1=inv[:], scalar2=-1.0,
                                            op0=ALU.mult, op1=ALU.mult)
                    naw = vec.tile([P, 1], F32, tag="naw")
                    nc.vector.tensor_mul(naw[:], wu[:, 0:1], na[:])

                    # deflate: C_new = C - alpha w w^T  (fused on DVE)
                    cnew = cpool.tile([P, P], F32, tag="C")
                    nc.vector.scalar_tensor_tensor(
                        out=cnew[:], in0=pwrep[:], scalar=naw[:],
                        in1=ccur[:], op0=ALU.mult, op1=ALU.add)

                    # W += (inv/sqrt(e)) w w^T  (off critical path)
                    rec_e = vec.tile([P, 1], F32, tag="rece")
                    nc.vector.reciprocal(rec_e[:], e_t[:])
                    sre = vec.tile([P, 1], F32, tag="sre")
                    nc.scalar.sqrt(sre[:], rec_e[:])
                    bw = vec.tile([P, 1], F32, tag="bw")
                    nc.vector.tensor_scalar(out=bw[:], in0=wu[:, 0:1],
                                            scalar1=inv[:], scalar2=sre[:],
                                            op0=ALU.mult, op1=ALU.mult)
                    nc.vector.scalar_tensor_tensor(
                        out=wacc[:], in0=pwrep[:], scalar=bw[:],
                        in1=wacc[:], op0=ALU.mult, op1=ALU.add)

                    ccur = cnew

            # ---------------- Phase C: apply pass ----------------
            with tc.tile_pool(name="xc", bufs=3) as xc_pool, \
                 tc.tile_pool(name="oc", bufs=3) as oc_pool, \
                 tc.tile_pool(name="xt", bufs=3) as xt_pool, \
                 tc.tile_pool(name="psT", bufs=3, space="PSUM") as psT, \
                 tc.tile_pool(name="psO", bufs=3, space="PSUM") as psO:
                for i in range(NSUP):
                    xsb = xc_pool.tile([P, BLKS, D], F32, tag="xin")
                    nc.sync.dma_start(
                        xsb[:],
                        xs_in[i * SUPER:(i + 1) * SUPER, :].rearrange(
                            "(n p) d -> p n d", p=P),
                    )
                    osb = oc_pool.tile([P, BLKS, D], F32, tag="oout")
                    for n in range(BLKS):
                        pT = psT.tile([P, P], F32, tag="pT")
                        nc.tensor.transpose(pT[:], xsb[:, n, :], ident[:])
                        xct = xt_pool.tile([P, P], F32, tag="xct")
                        nc.vector.tensor_scalar_sub(xct[:], pT[:], mu_col[:])
                        po = psO.tile([P, P], F32, tag="po")
                        nc.tensor.matmul(po[:], lhsT=xct[:], rhs=wacc[:],
                                         start=True, stop=True)
                        nc.scalar.copy(osb[:, n, :], po[:])
                    nc.sync.dma_start(
                        out_d[i * SUPER:(i + 1) * SUPER, :].rearrange(
                            "(n p) d -> p n d", p=P),
                        osb[:],
                    )
    nc.compile()
    return nc


_NC = None


def _get_nc():
    global _NC
    if _NC is None:
        _NC = build_nc()
    return _NC


def _ntff_hook():
    """Context manager driving NTFF profiling via the axon PJRT .so."""
    import ctypes
    import contextlib

    lib = ctypes.CDLL("/opt/axon/libaxon_pjrt.so")
    if not hasattr(lib, "axon_start_nrt_profile"):
        return None
    lib.axon_start_nrt_profile.argtypes = [
        ctypes.POINTER(ctypes.c_int64), ctypes.c_size_t]
    lib.axon_start_nrt_profile.restype = ctypes.c_int64
    lib.axon_stop_nrt_profile.argtypes = [ctypes.c_char_p]
    lib.axon_stop_nrt_profile.restype = ctypes.c_int64

    @contextlib.contextmanager
    def _hook(output_dir, device_ids):
        import jax
        jax.devices()
        if device_ids:
            ids = (ctypes.c_int64 * len(device_ids))(*device_ids)
            rc = lib.axon_start_nrt_profile(ids, len(device_ids))
        else:
            rc = lib.axon_start_nrt_profile(None, 0)
        if rc != 0:
            raise RuntimeError(f"axon_start_nrt_profile rc={rc}")
        try:
            yield
        finally:
            n = lib.axon_stop_nrt_profile(str(output_dir).encode())
            print(f"profile: {n} file(s) written to {output_dir}")

    return _hook


def run(x, R, trace=False, trace_kwargs=None):
    nc = _get_nc()
    x = np.ascontiguousarray(x, dtype=np.float32)
    R = np.ascontiguousarray(R, dtype=np.float32)
    in_maps = [
        {"xs": x[c * SHARD:(c + 1) * SHARD], "R": R} for c in range(NCORES)
    ]
    if not trace:
        res = bass_utils.run_bass_kernel_spmd(
            nc, in_maps, core_ids=list(range(NCORES)))
        out = np.concatenate([r["out"] for r in res.results], axis=0)
        return out, res

    # Custom trace path: the container's antenv lacks axon_hooks, so we
    # drive NTFF capture + perfetto conversion ourselves.
    import glob
    import tempfile
    from concourse import bass2jax
    import gauge.profiler
    from concourse._compat import FishPath

    hook = _ntff_hook()
    neff_dir = tempfile.mkdtemp(prefix="pw_prof_")
    with hook(neff_dir, [0]):
        results = bass2jax.run_bass_via_pjrt(nc, in_maps, n_cores=NCORES)
    out = np.concatenate([r["out"] for r in results], axis=0)
    ntffs = glob.glob(os.path.join(neff_dir, "*_body*.ntff"))
    if not ntffs:
        print(f"no NTFF produced in {neff_dir}:", os.listdir(neff_dir))
        return out, bass_utils.BassKernelResults(
            results=results, instructions_and_trace=None,
            profile_json=None, exec_time_ns=None)
    profile = gauge.profiler.Profile(
        profile_path=FishPath(neff_dir), kernel_dev_mode=True,
        profile_on_exit=False, bass_kernel=nc.m,
        offline_processing=True, fname="*_body*",
    )
    pr = profile.to_perfetto(model_index=(0,),
                             **(trace_kwargs or {}))[0]
    return out, bass_utils.BassKernelResults(
        results=results,
        instructions_and_trace=(pr.insts, pr.trace_path),
        profile_json=str(profile.json_path(0)),
        exec_time_ns=pr.exec_time_ns,
    )


def kernel(x, R):
    out, _ = run(x, R)
    return out
